# revision 7
# baseline (speedup 1.0000x reference)
# Causal self-attention kernel for 8 Trainium2 NeuronCores (Bass/Tile).
#
# Sharding: core c -> batch b = c//4, head group g = c%4 (heads 4g..4g+3).
# Each core computes the qkv projection for its batch restricted to its heads
# (column-sharded Wqkv), rope, causal flash attention for its 4 heads, and a
# row-sharded output projection producing a partial [S, D] bf16 output.  The
# host sums the 4 partials per batch (f32) and adds bout.
#
# Device-side notes:
#  * All matmul inputs are bf16 (fp32 matmul is 4x slower on the PE); all
#    accumulation is f32 in PSUM.
#  * x is pre-transposed on host to xT [D, S] so the contraction dim lands on
#    SBUF partitions without any on-device transpose.
#  * q/k are produced directly transposed (qT/kT [dims, S]).  Within each head
#    the dims are permuted to [evens(32), odds(32)] so rope becomes
#    rot = x*P + swap32(x)*Q with host-built per-row tables P/Q; swap32 is two
#    32-partition-block exchanges done with SBUF->SBUF DMAs.
#  * Attention runs pair-processed: heads (2p, 2p+1) share the j (k-tile)
#    loop.  Scores are computed transposed, sT[k, q], as K=64 matmuls using
#    PE row tiling (tile_position (0,0) / (64,0)): both heads' score matmuls
#    execute CONCURRENTLY in the two 64-row halves of the PE array, so the
#    pair costs the same PE time as one zero-padded K=128 matmul.
#  * Causal masking of diagonal score tiles is one extra accumulating matmul
#    (-30000 * upper-triangle) per head -- no cross-engine dependency.
#  * Softmax without max-subtraction (scores ~ N(0,1); exp is safe in f32):
#    p = exp(s/8) straight out of PSUM on the scalar engine, bf16 out, one
#    ACT call covering both heads' 512-col blocks.
#  * v_ext [k, 65] carries a ones-column so the PV matmul accumulates the
#    softmax denominator as row 64 of oT [65, q].  oT is evacuated to SBUF
#    immediately; reciprocal_approx_fast + gpsimd partition_broadcast +
#    multiply run off the PE's critical path.
#  * The whole kernel is ONE interleaved instruction stream: projection
#    groups (q23/k23/v), later-half v groups, and the output projection are
#    emitted as "inserts" inside the attention piece loop, so the PE never
#    idles while the scalar engine chews exp, and exp starts ~35us earlier.
#  * PSUM: s-pool 2 x [128,1024] (4 banks) + oT 2 x [65,1024] (4 banks) = 8.
#    Projection groups and y tiles allocate from the s-pool.
#  * Output projection: y[q, n] = sum_d aT[d, q] * Wout[d, n], q-tiled;
#    y stored bf16 (halves the output write traffic).

import numpy as np
import ml_dtypes

import concourse.bass as bass
import concourse.tile as tile
import concourse.mybir as mybir
from concourse import bacc
from concourse.bass import ts, ds
from concourse.bass_utils import run_bass_kernel_spmd

BF16 = mybir.dt.bfloat16
F32 = mybir.dt.float32
AF = mybir.ActivationFunctionType
ALU = mybir.AluOpType

B, S, D = 2, 2048, 1024
H, HD = 16, 64
NCORES = 8
HPC = 4            # heads per core
NT = S // 128      # 16 seq tiles
SCALE = HD ** -0.5
MASK_NEG = -30000.0

# Module-level knobs / results (used by test.py).
TRACE = False
LAST_RESULTS = None


def _body(ctx, tc, ins, outs):
    nc = tc.nc
    xT, wqk, wv, wout, bqk, bvb, ropeP, ropeQ = ins
    (y,) = outs

    # ---- SBUF pools ----
    p_x = ctx.enter_context(tc.tile_pool(name="x", bufs=1))
    p_w = ctx.enter_context(tc.tile_pool(name="w", bufs=1))
    p_cst = ctx.enter_context(tc.tile_pool(name="cst", bufs=1))
    p_qk = ctx.enter_context(tc.tile_pool(name="qk", bufs=1))
    p_vx = ctx.enter_context(tc.tile_pool(name="vx", bufs=1))
    p_aT = ctx.enter_context(tc.tile_pool(name="aT", bufs=1))
    p_tmp = ctx.enter_context(tc.tile_pool(name="tmp", bufs=3))
    p_p = ctx.enter_context(tc.tile_pool(name="p", bufs=6))
    p_r = ctx.enter_context(tc.tile_pool(name="r", bufs=2))
    p_y = ctx.enter_context(tc.tile_pool(name="y", bufs=3))

    # ---- load inputs (x and weight DMAs interleaved) ----
    x_sb, wqk_sb, wv_sb = [], [], []
    for kc in range(8):
        t = p_x.tile([128, S], BF16, tag=f"x{kc}", name=f"x{kc}")
        # issue x loads from the scalar engine's DMA queue: the sync
        # sequencer alone needs ~17us just to issue all input DMAs
        nc.scalar.dma_start(t[:, :], xT[ts(kc, 128), :])
        x_sb.append(t)
        t = p_w.tile([128, 512], BF16, tag=f"wqk{kc}", name=f"wqk{kc}")
        nc.sync.dma_start(t[:, :], wqk[ts(kc, 128), :])
        wqk_sb.append(t)
        t = p_w.tile([128, 256], BF16, tag=f"wv{kc}", name=f"wv{kc}")
        nc.sync.dma_start(t[:, :], wv[ts(kc, 128), :])
        wv_sb.append(t)
    wout_sb = []
    for kc in range(2):
        t = p_w.tile([128, D], BF16, tag=f"wout{kc}", name=f"wout{kc}")
        nc.sync.dma_start(t[:, :], wout[ts(kc, 128), :])
        wout_sb.append(t)
    tabP = p_cst.tile([128, S], BF16, tag="tabP")
    nc.sync.dma_start(tabP[:, :], ropeP[:, :])
    tabQ = p_cst.tile([128, S], BF16, tag="tabQ")
    nc.sync.dma_start(tabQ[:, :], ropeQ[:, :])
    bqk_sb = []
    for mc in range(4):
        t = p_cst.tile([128, 1], F32, tag=f"bqk{mc}", name=f"bqk{mc}")
        nc.sync.dma_start(t[:, :], bqk[ts(mc, 128)].rearrange("(p o) -> p o", o=1))
        bqk_sb.append(t)
    bvb_sb = p_cst.tile([128, 256], F32, tag="bvb")
    nc.sync.dma_start(bvb_sb[:, :], bvb[:, :])

    # constants for the matmul-based causal mask of diagonal score tiles:
    # s_diag += (neg_ident.T @ upper01) = -30000 where k > q.
    # (only is_ge is implemented for affine_select in this compiler)
    ones_t = p_cst.tile([128, 128], BF16, tag="ones")
    nc.vector.memset(ones_t[:, :], 1.0)
    upper01 = p_cst.tile([128, 128], BF16, tag="upper01")
    nc.gpsimd.affine_select(upper01[:, :], ones_t[:, :], pattern=[[-1, 128]],
                            compare_op=ALU.is_ge, fill=0.0, base=-1,
                            channel_multiplier=1)   # keep where k - q - 1 >= 0
    lower_t = p_cst.tile([128, 128], BF16, tag="lower")
    nc.gpsimd.affine_select(lower_t[:, :], ones_t[:, :], pattern=[[1, 128]],
                            compare_op=ALU.is_ge, fill=0.0, base=0,
                            channel_multiplier=-1)  # keep where q - k >= 0
    ident_t = p_cst.tile([128, 128], BF16, tag="ident")
    nc.gpsimd.affine_select(ident_t[:, :], lower_t[:, :], pattern=[[-1, 128]],
                            compare_op=ALU.is_ge, fill=0.0, base=0,
                            channel_multiplier=1)   # and k - q >= 0
    neg_ident = p_cst.tile([128, 128], BF16, tag="neg_ident")
    nc.vector.tensor_scalar_mul(neg_ident[:, :], ident_t[:, :], MASK_NEG)

    # [q01, q23, k01, k23], bf16 [128, S] each (post-rope); head pair p has
    # its even head on partitions 0-63 and odd head on 64-127.
    qk_sb = [p_qk.tile([128, S], BF16, tag=f"qkT{mc}", name=f"qkT{mc}")
             for mc in range(4)]
    vx_sb = [None] * NT  # [128, 4*65] bf16: per head 64 v-cols + ones col
    aT_sb = [p_aT.tile([128, S], BF16, tag=f"aT{i}", name=f"aT{i}")
             for i in range(2)]

    # ---- one shared PSUM budget for the whole kernel ----
    # ps_s: everything that is evacuated quickly (scores, projections, y)
    # ps_o: the two long-lived PV accumulators of the current head pair
    with tc.tile_pool(name="ps_s", bufs=2, space="PSUM") as ps_s, \
         tc.tile_pool(name="ps_o", bufs=2, space="PSUM") as ps_o:

        with nc.named_scope("warmup"):
            # ~4us of dense K=128 matmuls (discarded) flips the PE clock
            # gate to 2.4 GHz while the x DMAs are still in flight
            wu = ps_s.tile([128, 1024], F32, tag="s", name="warmup")
            for r in range(18):
                nc.tensor.matmul(wu[:, 0:512], wqk_sb[0][:, 0:128],
                                 wqk_sb[0][:, :], start=(r == 0),
                                 stop=(r == 17), skip_group_check=True)

        def rope_evac(mc, ns, qk_ps):
            # qk_ps: [128, 512] f32 PSUM view -> qk_sb[mc][:, ns*512:...]
            raw = p_tmp.tile([128, 512], BF16, tag="raw")
            nc.vector.tensor_scalar_add(raw[:, :], qk_ps, bqk_sb[mc][:, :])
            # swap32: exchange adjacent 32-partition blocks (scalar/sync
            # queues split the DMA issue cost)
            swp = p_tmp.tile([128, 512], BF16, tag="swp")
            swap_q = nc.scalar if mc < 2 else nc.sync
            for blk, sb in ((0, 32), (32, 0), (64, 96), (96, 64)):
                swap_q.dma_start(swp[blk:blk + 32, :], raw[sb:sb + 32, :])
            t1 = p_tmp.tile([128, 512], BF16, tag="t1")
            nc.vector.tensor_mul(t1[:, :], swp[:, :], tabQ[:, ts(ns, 512)])
            t2 = p_tmp.tile([128, 512], BF16, tag="t2")
            nc.vector.tensor_mul(t2[:, :], raw[:, :], tabP[:, ts(ns, 512)])
            nc.vector.tensor_add(qk_sb[mc][:, ts(ns, 512)], t1[:, :], t2[:, :])

        def proj_qk_half(mc, nh):
            # projection of qk group mc for seq half nh (2 x 512 columns)
            t = ps_s.tile([128, 1024], F32, tag="s", name=f"qk{mc}_{nh}")
            for kc in range(8):
                for ns2 in range(2):
                    nc.tensor.matmul(
                        t[:, ts(ns2, 512)],
                        wqk_sb[kc][:, ts(mc, 128)],
                        x_sb[kc][:, ds(1024 * nh + 512 * ns2, 512)],
                        start=(kc == 0), stop=(kc == 7))
            for ns2 in range(2):
                rope_evac(mc, 2 * nh + ns2, t[:, ts(ns2, 512)])

        def proj_v4(vg):
            # v projection for seq tiles 4vg .. 4vg+3
            t = ps_s.tile([128, 1024], F32, tag="s", name=f"v{vg}")
            for st4 in range(4):
                st = 4 * vg + st4
                for kc in range(8):
                    nc.tensor.matmul(
                        t[:, ts(st4, 256)],
                        x_sb[kc][:, ts(st, 128)],
                        wv_sb[kc][:, :],
                        start=(kc == 0), stop=(kc == 7))
            for st4 in range(4):
                st = 4 * vg + st4
                vx_t = p_vx.tile([128, HPC * 65], BF16, tag=f"vx{st}",
                                 name=f"vx{st}")
                vv = vx_t.rearrange("p (h c) -> p h c", c=65)
                nc.vector.memset(vv[:, :, 64:65], 1.0)
                nc.vector.tensor_add(
                    vv[:, :, 0:64],
                    t.rearrange("p (g h c) -> p g h c", g=4, c=64)[:, st4, :, :],
                    bvb_sb.rearrange("p (h c) -> p h c", c=64)[:, :, :])
                vx_sb[st] = vx_t

        def proj_y(qt):
            # output projection for seq tile qt; y tile borrows the s pool
            t = ps_s.tile([128, 1024], F32, tag="s", name=f"y{qt}")
            for nh in range(2):
                for kc in range(2):
                    nc.tensor.matmul(
                        t[:, ts(nh, 512)],
                        aT_sb[kc][:, ts(qt, 128)],
                        wout_sb[kc][:, ts(nh, 512)],
                        start=(kc == 0), stop=(kc == 1))
            y_sb = p_y.tile([128, 1024], BF16, tag="ysb")
            nc.vector.tensor_copy(y_sb[:, 0:512], t[:, 0:512])
            nc.scalar.copy(y_sb[:, 512:1024], t[:, 512:1024])
            nc.sync.dma_start(y[ts(qt, 128), :], y_sb[:, :])

        # ---- attention machinery ----
        pend = [None]      # previous piece awaiting its PV matmuls
        finishes = []      # deferred oT -> aT normalizations

        def emit_pv(pv):
            p_t, pair, half, j, b, t0, cw, oTA, oTB = pv
            qlo = 8 * half
            off = (t0 - qlo) * 128
            jlast = qlo + 4 * b + 3
            nc.tensor.matmul(
                oTA[:, ds(off, cw)],
                vx_sb[j][:, ds(65 * (2 * pair), 65)],
                p_t[:, 0:cw],
                start=(j == 0), stop=(j == jlast), skip_group_check=True)
            nc.tensor.matmul(
                oTB[:, ds(off, cw)],
                vx_sb[j][:, ds(65 * (2 * pair + 1), 65)],
                p_t[:, ds(512, cw)],
                start=(j == 0), stop=(j == jlast), skip_group_check=True)

        def finish_half(h, hp, hr, half, oT, csz):
            # Evacuate oT to SBUF right away so the PSUM banks free without
            # waiting on the reciprocal; then normalize from the copy in
            # chunks, fully off the PE's path.
            num = p_r.tile([64, 1024], BF16, tag="num", name=f"num{h}_{half}")
            nc.vector.tensor_copy(num[:, :], oT[0:64, :])
            den = p_r.tile([1, 1024], F32, tag="den", name=f"den{h}_{half}")
            nc.vector.tensor_copy(den[:, :], oT[64:65, :])
            for c in range(1024 // csz):
                r_t = p_r.tile([1, 512], F32, tag="r", name=f"r{h}_{half}_{c}")
                nc.vector.reciprocal_approx_fast(r_t[:, 0:csz],
                                                 den[:, ds(csz * c, csz)])
                rb_t = p_r.tile([64, 512], F32, tag="rb",
                                name=f"rb{h}_{half}_{c}")
                nc.gpsimd.partition_broadcast(rb_t[:, 0:csz], r_t[:, 0:csz])
                nc.vector.tensor_mul(
                    aT_sb[hp][64 * hr:64 * hr + 64,
                              ds(1024 * half + csz * c, csz)],
                    num[:, ds(csz * c, csz)], rb_t[:, 0:csz])

        def flush_pend(inserts):
            if pend[0] is not None:
                emit_pv(pend[0])
                pend[0] = None
                for fin in finishes:
                    fin()
                finishes.clear()
            if inserts:
                ins_fn = inserts.pop(0)
                if ins_fn is not None:
                    ins_fn()

        def attn_group(pair, half, inserts, last_csz=512):
            qT, kT = qk_sb[pair], qk_sb[2 + pair]
            qlo, qhi = 8 * half, 8 * half + 8
            with nc.named_scope(f"attn_p{pair}h{half}"):
                oTA = ps_o.tile([65, 1024], F32, tag="oT",
                                name=f"oT{pair}_{half}A")
                oTB = ps_o.tile([65, 1024], F32, tag="oT",
                                name=f"oT{pair}_{half}B")
                for j in range(qhi):
                    for bb in range(2):
                        if j >= qlo + 4 * bb + 4:
                            continue
                        t0 = max(j, qlo + 4 * bb)
                        cw = (qlo + 4 * bb + 4 - t0) * 128
                        diag = (j >= qlo + 4 * bb)
                        s_t = ps_s.tile([128, 1024], F32, tag="s",
                                        name=f"s{pair}_{half}_{j}_{bb}")
                        # row-tiled K=64 scores: both heads concurrently
                        nc.tensor.matmul(
                            s_t[:, 0:cw], kT[0:64, ts(j, 128)],
                            qT[0:64, ds(t0 * 128, cw)],
                            start=True, stop=not diag, skip_group_check=True)
                        nc.tensor.matmul(
                            s_t[:, ds(512, cw)], kT[64:128, ts(j, 128)],
                            qT[64:128, ds(t0 * 128, cw)],
                            start=True, stop=not diag, skip_group_check=True)
                        if diag:
                            nc.tensor.matmul(
                                s_t[:, 0:128], neg_ident[:, :], upper01[:, :],
                                start=False, stop=True, skip_group_check=True)
                            nc.tensor.matmul(
                                s_t[:, 512:640], neg_ident[:, :],
                                upper01[:, :],
                                start=False, stop=True, skip_group_check=True)
                        p_t = p_p.tile([128, 1024], BF16, tag="p")
                        if cw == 512:
                            nc.scalar.activation(p_t[:, :], s_t[:, :], AF.Exp,
                                                 scale=SCALE)
                        else:
                            sv = s_t.rearrange("p (b n) -> p b n", n=512)
                            pv_ = p_t.rearrange("p (b n) -> p b n", n=512)
                            nc.scalar.activation(pv_[:, :, 0:cw],
                                                 sv[:, :, 0:cw], AF.Exp,
                                                 scale=SCALE)
                        flush_pend(inserts)
                        pend[0] = (p_t, pair, half, j, bb, t0, cw, oTA, oTB)
                for hr in range(2):
                    h = 2 * pair + hr
                    oT = oTA if hr == 0 else oTB
                    finishes.append(
                        lambda h=h, hp=pair, hr=hr, half=half, oT=oT,
                               csz=last_csz:
                            finish_half(h, hp, hr, half, oT, csz))

        # ---- emission: one interleaved stream ----
        with nc.named_scope("qk_proj_head"):
            for nh in range(2):
                proj_qk_half(0, nh)    # q01
            for nh in range(2):
                proj_qk_half(2, nh)    # k01
            proj_v4(0)                 # vx[0..3]
        # pair 0 half 0; meanwhile project q23/k23 + vx[4..7]
        g1 = [lambda: proj_v4(1),
              lambda: proj_qk_half(1, 0), lambda: proj_qk_half(1, 1),
              lambda: proj_qk_half(3, 0), lambda: proj_qk_half(3, 1)]
        attn_group(0, 0, g1)
        # pair 1 half 0; meanwhile project vx[8..15] for the second halves
        g2 = [lambda: proj_v4(2), lambda: proj_v4(3)]
        attn_group(1, 0, g2)
        # pair 0 half 1; meanwhile output projection of the finished half 0
        g3 = [None, None] + [lambda qt=qt: proj_y(qt) for qt in range(8)]
        attn_group(0, 1, g3)
        # pair 1 half 1
        attn_group(1, 1, [], last_csz=256)
        flush_pend([])
        for fin in finishes:
            fin()

        # ---- tail: output projection of half 1 ----
        with nc.named_scope("y_proj_tail"):
            for qt in range(8, NT):
                proj_y(qt)


def build():
    nc = bacc.Bacc("TRN2", target_bir_lowering=False, debug=False,
                   num_devices=NCORES)
    xT = nc.dram_tensor("xT", [D, S], BF16, kind="ExternalInput").ap()
    wqk = nc.dram_tensor("wqk", [D, 512], BF16, kind="ExternalInput").ap()
    wv = nc.dram_tensor("wv", [D, 256], BF16, kind="ExternalInput").ap()
    wout = nc.dram_tensor("wout", [256, D], BF16, kind="ExternalInput").ap()
    bqk = nc.dram_tensor("bqk", [512], F32, kind="ExternalInput").ap()
    bvb = nc.dram_tensor("bvb", [128, 256], F32, kind="ExternalInput").ap()
    ropeP = nc.dram_tensor("ropeP", [128, S], BF16, kind="ExternalInput").ap()
    ropeQ = nc.dram_tensor("ropeQ", [128, S], BF16, kind="ExternalInput").ap()
    y = nc.dram_tensor("y", [S, D], BF16, kind="ExternalOutput").ap()

    from contextlib import ExitStack
    with tile.TileContext(nc) as tc:
        with ExitStack() as ctx:
            _body(ctx, tc, (xT, wqk, wv, wout, bqk, bvb, ropeP, ropeQ), (y,))
    nc.compile()
    return nc


_EVEN_ODD = np.concatenate([np.arange(0, HD, 2), np.arange(1, HD, 2)])


def make_core_inputs(x, rope_cos, rope_sin, Wqkv, bqkv, Wout, bout, core):
    """Build the per-core device input map (numpy, host-side sharding)."""
    b, g = core // HPC, core % HPC
    heads = [HPC * g + i for i in range(HPC)]
    bf = ml_dtypes.bfloat16

    xT = np.ascontiguousarray(x[b].T).astype(bf)

    # wqk columns: [q01, q23, k01, k23]; within each head [evens, odds]
    qcols, kcols = [], []
    for h in heads:
        qcols.append(Wqkv[:, 0 * D + 64 * h + _EVEN_ODD])
        kcols.append(Wqkv[:, 1 * D + 64 * h + _EVEN_ODD])
    wqk_np = np.concatenate(
        [qcols[0], qcols[1], qcols[2], qcols[3],
         kcols[0], kcols[1], kcols[2], kcols[3]], axis=1)
    bq = [bqkv[0 * D + 64 * h + _EVEN_ODD] for h in heads]
    bk = [bqkv[1 * D + 64 * h + _EVEN_ODD] for h in heads]
    bqk_np = np.concatenate([bq[0], bq[1], bq[2], bq[3],
                             bk[0], bk[1], bk[2], bk[3]])

    wv_np = np.concatenate(
        [Wqkv[:, 2 * D + 64 * h:2 * D + 64 * h + 64] for h in heads], axis=1)
    bv = np.concatenate(
        [bqkv[2 * D + 64 * h:2 * D + 64 * h + 64] for h in heads])
    bvb_np = np.tile(bv[None, :], (128, 1)).astype(np.float32)

    wout_np = np.concatenate(
        [Wout[64 * h:64 * h + 64, :] for h in heads], axis=0)

    cosT = np.ascontiguousarray(rope_cos.T).astype(np.float32)  # [32, S]
    sinT = np.ascontiguousarray(rope_sin.T).astype(np.float32)
    ropeP_np = np.tile(np.concatenate([cosT, cosT], axis=0), (2, 1))
    ropeQ_np = np.tile(np.concatenate([-sinT, sinT], axis=0), (2, 1))

    return {
        "xT": xT,
        "wqk": np.ascontiguousarray(wqk_np).astype(bf),
        "wv": np.ascontiguousarray(wv_np).astype(bf),
        "wout": np.ascontiguousarray(wout_np).astype(bf),
        "bqk": bqk_np.astype(np.float32),
        "bvb": bvb_np,
        "ropeP": np.ascontiguousarray(ropeP_np).astype(bf),
        "ropeQ": np.ascontiguousarray(ropeQ_np).astype(bf),
    }


_NC_CACHE = None


def kernel(x, rope_cos, rope_sin, Wqkv, bqkv, Wout, bout):
    global _NC_CACHE, LAST_RESULTS
    x = np.asarray(x, dtype=np.float32)
    rope_cos = np.asarray(rope_cos, dtype=np.float32)
    rope_sin = np.asarray(rope_sin, dtype=np.float32)
    Wqkv = np.asarray(Wqkv, dtype=np.float32)
    bqkv = np.asarray(bqkv, dtype=np.float32)
    Wout = np.asarray(Wout, dtype=np.float32)
    bout = np.asarray(bout, dtype=np.float32)

    if _NC_CACHE is None:
        _NC_CACHE = build()
    nc = _NC_CACHE

    in_maps = [
        make_core_inputs(x, rope_cos, rope_sin, Wqkv, bqkv, Wout, bout, c)
        for c in range(NCORES)
    ]
    res = run_bass_kernel_spmd(nc, in_maps, core_ids=list(range(NCORES)),
                               trace=TRACE)
    LAST_RESULTS = res

    out = np.zeros((B, S, D), dtype=np.float32)
    for c in range(NCORES):
        out[c // HPC] += np.asarray(res.results[c]["y"]).astype(np.float32)
    out += bout[None, None, :]
    return out


# revision 9
# speedup vs baseline: 1.0528x; 1.0528x over previous
# Causal self-attention kernel for 8 Trainium2 NeuronCores (Bass/Tile).
#
# Sharding: core c -> batch b = c//4, head group g = c%4 (heads 4g..4g+3).
# Each core computes the qkv projection for its batch restricted to its heads
# (column-sharded Wqkv), rope, causal flash attention for its 4 heads, and a
# row-sharded output projection producing a partial [S, D] bf16 output.  The
# host sums the 4 partials per batch (f32) and adds bout.
#
# Device-side notes:
#  * All matmul inputs are bf16 (fp32 matmul is 4x slower on the PE); all
#    accumulation is f32 in PSUM.
#  * x is pre-transposed on host to xT [D, S] so the contraction dim lands on
#    SBUF partitions without any on-device transpose.
#  * q/k are produced directly transposed (qT/kT [dims, S]).  Within each head
#    the dims are permuted to [evens(32), odds(32)] so rope becomes
#    rot = x*P + swap32(x)*Q with host-built per-row tables P/Q; swap32 is two
#    32-partition-block exchanges done with SBUF->SBUF DMAs.
#  * Attention runs pair-processed in q-QUARTERS (512 cols): heads (2p, 2p+1)
#    share the j (k-tile) loop.  Scores are computed transposed, sT[k, q], as
#    K=64 matmuls using PE row tiling (tile_position (0,0) / (64,0)): both
#    heads' score matmuls execute CONCURRENTLY in the two 64-row halves of
#    the PE array, costing the same PE time as one matmul.
#  * Causal masking of diagonal score tiles is one extra accumulating matmul
#    (-30000 * upper-triangle) per head -- no cross-engine dependency.
#  * Softmax without max-subtraction (scores ~ N(0,1); exp is safe in f32):
#    p = exp(s/8) straight out of PSUM on the scalar engine, bf16 out, one
#    ACT call covering both heads' 512-col blocks.
#  * v_ext [k, 65] carries a ones-column so the PV matmul accumulates the
#    softmax denominator as row 64 of oT [65, 512].  oT is evacuated to SBUF
#    immediately; reciprocal_approx_fast + gpsimd partition_broadcast +
#    multiply run off the PE's critical path.
#  * The whole kernel is ONE interleaved instruction stream: projection
#    groups (q23/k23/v) and the output projection are emitted as "inserts"
#    inside the attention piece loop, so the PE never idles while the scalar
#    engine chews exp, and exp starts ~35us earlier.  PSUM: scores pool
#    2 x [128,1024] (4 banks) + oT 2 x [65,512] (2 banks) + a dedicated
#    projection/y pool 1 x [128,1024] (2 banks) = 8 banks.
#  * Output projection: y[q, n] = sum_d aT[d, q] * Wout[d, n], q-tiled;
#    y stored bf16 (halves the output write traffic).  Only y(12..15) remain
#    after the last attention quarter -> short tail.

import numpy as np
import ml_dtypes

import concourse.bass as bass
import concourse.tile as tile
import concourse.mybir as mybir
from concourse import bacc
from concourse.bass import ts, ds
from concourse.bass_utils import run_bass_kernel_spmd

BF16 = mybir.dt.bfloat16
F32 = mybir.dt.float32
AF = mybir.ActivationFunctionType
ALU = mybir.AluOpType

B, S, D = 2, 2048, 1024
H, HD = 16, 64
NCORES = 8
HPC = 4            # heads per core
NT = S // 128      # 16 seq tiles
SCALE = HD ** -0.5
MASK_NEG = -30000.0

# Module-level knobs / results (used by test.py).
TRACE = False
LAST_RESULTS = None


def _body(ctx, tc, ins, outs):
    nc = tc.nc
    xT, wqk, wv, wout, bqk, bvb, ropeP, ropeQ = ins
    (y,) = outs

    # ---- SBUF pools ----
    p_x = ctx.enter_context(tc.tile_pool(name="x", bufs=1))
    p_w = ctx.enter_context(tc.tile_pool(name="w", bufs=1))
    p_cst = ctx.enter_context(tc.tile_pool(name="cst", bufs=1))
    p_qk = ctx.enter_context(tc.tile_pool(name="qk", bufs=1))
    p_vx = ctx.enter_context(tc.tile_pool(name="vx", bufs=1))
    p_aT = ctx.enter_context(tc.tile_pool(name="aT", bufs=1))
    p_tmp = ctx.enter_context(tc.tile_pool(name="tmp", bufs=3))
    p_p = ctx.enter_context(tc.tile_pool(name="p", bufs=6))
    p_r = ctx.enter_context(tc.tile_pool(name="r", bufs=2))
    p_y = ctx.enter_context(tc.tile_pool(name="y", bufs=3))

    # ---- load inputs ----
    # x chunks (scalar queue) and wqk chunks (sync queue) interleaved so the
    # projection's kc-loop can start as chunks land; small/late tensors go
    # via the gpsimd queue so their issue cost doesn't delay the big ones.
    x_sb, wqk_sb, wv_sb = [], [], []
    for kc in range(8):
        t = p_w.tile([128, 512], BF16, tag=f"wqk{kc}", name=f"wqk{kc}")
        nc.sync.dma_start(t[:, :], wqk[ts(kc, 128), :])
        wqk_sb.append(t)
        t = p_x.tile([128, S], BF16, tag=f"x{kc}", name=f"x{kc}")
        nc.scalar.dma_start(t[:, :], xT[ts(kc, 128), :])
        x_sb.append(t)
    bqk_sb = []
    for mc in range(4):
        t = p_cst.tile([128, 1], F32, tag=f"bqk{mc}", name=f"bqk{mc}")
        nc.gpsimd.dma_start(t[:, :], bqk[ts(mc, 128)].rearrange("(p o) -> p o", o=1))
        bqk_sb.append(t)
    tabP = p_cst.tile([128, S], BF16, tag="tabP")
    nc.gpsimd.dma_start(tabP[:, :], ropeP[:, :])
    tabQ = p_cst.tile([128, S], BF16, tag="tabQ")
    nc.gpsimd.dma_start(tabQ[:, :], ropeQ[:, :])
    bvb_sb = p_cst.tile([128, 256], F32, tag="bvb")
    nc.gpsimd.dma_start(bvb_sb[:, :], bvb[:, :])
    for kc in range(8):
        t = p_w.tile([128, 256], BF16, tag=f"wv{kc}", name=f"wv{kc}")
        nc.gpsimd.dma_start(t[:, :], wv[ts(kc, 128), :])
        wv_sb.append(t)
    wout_sb = []
    for kc in range(2):
        t = p_w.tile([128, D], BF16, tag=f"wout{kc}", name=f"wout{kc}")
        nc.gpsimd.dma_start(t[:, :], wout[ts(kc, 128), :])
        wout_sb.append(t)

    # constants for the matmul-based causal mask of diagonal score tiles:
    # s_diag += (neg_ident.T @ upper01) = -30000 where k > q.
    ones_t = p_cst.tile([128, 128], BF16, tag="ones")
    nc.vector.memset(ones_t[:, :], 1.0)
    upper01 = p_cst.tile([128, 128], BF16, tag="upper01")
    nc.gpsimd.affine_select(upper01[:, :], ones_t[:, :], pattern=[[-1, 128]],
                            compare_op=ALU.is_ge, fill=0.0, base=-1,
                            channel_multiplier=1)   # keep where k - q - 1 >= 0
    lower_t = p_cst.tile([128, 128], BF16, tag="lower")
    nc.gpsimd.affine_select(lower_t[:, :], ones_t[:, :], pattern=[[1, 128]],
                            compare_op=ALU.is_ge, fill=0.0, base=0,
                            channel_multiplier=-1)  # keep where q - k >= 0
    ident_t = p_cst.tile([128, 128], BF16, tag="ident")
    nc.gpsimd.affine_select(ident_t[:, :], lower_t[:, :], pattern=[[-1, 128]],
                            compare_op=ALU.is_ge, fill=0.0, base=0,
                            channel_multiplier=1)   # and k - q >= 0
    neg_ident = p_cst.tile([128, 128], BF16, tag="neg_ident")
    nc.vector.tensor_scalar_mul(neg_ident[:, :], ident_t[:, :], MASK_NEG)

    # [q01, q23, k01, k23], bf16 [128, S] each (post-rope); head pair p has
    # its even head on partitions 0-63 and odd head on 64-127.
    qk_sb = [p_qk.tile([128, S], BF16, tag=f"qkT{mc}", name=f"qkT{mc}")
             for mc in range(4)]
    vx_sb = [None] * NT  # [128, 4*65] bf16: per head 64 v-cols + ones col
    aT_sb = [p_aT.tile([128, S], BF16, tag=f"aT{i}", name=f"aT{i}")
             for i in range(2)]

    # ---- PSUM: 4 (scores) + 2 (oT) + 2 (proj/y) = 8 banks ----
    with tc.tile_pool(name="ps_s", bufs=2, space="PSUM") as ps_s, \
         tc.tile_pool(name="ps_o", bufs=2, space="PSUM") as ps_o, \
         tc.tile_pool(name="ps_m", bufs=1, space="PSUM") as ps_m:

        with nc.named_scope("warmup"):
            # ~4us of dense K=128 matmuls (discarded) flips the PE clock
            # gate to 2.4 GHz while the x DMAs are still in flight
            wu = ps_m.tile([128, 1024], F32, tag="m", name="warmup")
            for r in range(18):
                nc.tensor.matmul(wu[:, 0:512], wqk_sb[0][:, 0:128],
                                 wqk_sb[0][:, :], start=(r == 0),
                                 stop=(r == 17), skip_group_check=True)

        def rope_evac(mc, ns, qk_ps):
            # qk_ps: [128, 512] f32 PSUM view -> qk_sb[mc][:, ns*512:...]
            raw = p_tmp.tile([128, 512], BF16, tag="raw")
            nc.vector.tensor_scalar_add(raw[:, :], qk_ps, bqk_sb[mc][:, :])
            # swap32: exchange adjacent 32-partition blocks (scalar/sync
            # queues split the DMA issue cost)
            swp = p_tmp.tile([128, 512], BF16, tag="swp")
            swap_q = nc.scalar if mc < 2 else nc.sync
            for blk, sb in ((0, 32), (32, 0), (64, 96), (96, 64)):
                swap_q.dma_start(swp[blk:blk + 32, :], raw[sb:sb + 32, :])
            t1 = p_tmp.tile([128, 512], BF16, tag="t1")
            nc.vector.tensor_mul(t1[:, :], swp[:, :], tabQ[:, ts(ns, 512)])
            t2 = p_tmp.tile([128, 512], BF16, tag="t2")
            nc.vector.tensor_mul(t2[:, :], raw[:, :], tabP[:, ts(ns, 512)])
            nc.vector.tensor_add(qk_sb[mc][:, ts(ns, 512)], t1[:, :], t2[:, :])

        def proj_qk_half(mc, nh):
            # projection of qk group mc for seq half nh (2 x 512 columns)
            t = ps_m.tile([128, 1024], F32, tag="m", name=f"qk{mc}_{nh}")
            for kc in range(8):
                for ns2 in range(2):
                    nc.tensor.matmul(
                        t[:, ts(ns2, 512)],
                        wqk_sb[kc][:, ts(mc, 128)],
                        x_sb[kc][:, ds(1024 * nh + 512 * ns2, 512)],
                        start=(kc == 0), stop=(kc == 7))
            for ns2 in range(2):
                rope_evac(mc, 2 * nh + ns2, t[:, ts(ns2, 512)])

        def proj_v4(vg):
            # v projection for seq tiles 4vg .. 4vg+3
            t = ps_m.tile([128, 1024], F32, tag="m", name=f"v{vg}")
            for st4 in range(4):
                st = 4 * vg + st4
                for kc in range(8):
                    nc.tensor.matmul(
                        t[:, ts(st4, 256)],
                        x_sb[kc][:, ts(st, 128)],
                        wv_sb[kc][:, :],
                        start=(kc == 0), stop=(kc == 7))
            for st4 in range(4):
                st = 4 * vg + st4
                vx_t = p_vx.tile([128, HPC * 65], BF16, tag=f"vx{st}",
                                 name=f"vx{st}")
                vv = vx_t.rearrange("p (h c) -> p h c", c=65)
                nc.vector.memset(vv[:, :, 64:65], 1.0)
                nc.vector.tensor_add(
                    vv[:, :, 0:64],
                    t.rearrange("p (g h c) -> p g h c", g=4, c=64)[:, st4, :, :],
                    bvb_sb.rearrange("p (h c) -> p h c", c=64)[:, :, :])
                vx_sb[st] = vx_t

        def proj_y(qt):
            # output projection for seq tile qt
            t = ps_m.tile([128, 1024], F32, tag="m", name=f"y{qt}")
            for nh in range(2):
                for kc in range(2):
                    nc.tensor.matmul(
                        t[:, ts(nh, 512)],
                        aT_sb[kc][:, ts(qt, 128)],
                        wout_sb[kc][:, ts(nh, 512)],
                        start=(kc == 0), stop=(kc == 1))
            y_sb = p_y.tile([128, 1024], BF16, tag="ysb")
            nc.vector.tensor_copy(y_sb[:, 0:512], t[:, 0:512])
            nc.scalar.copy(y_sb[:, 512:1024], t[:, 512:1024])
            nc.sync.dma_start(y[ts(qt, 128), :], y_sb[:, :])

        # ---- attention machinery ----
        pend = [None]      # previous piece awaiting its PV matmuls
        finishes = []      # deferred oT -> aT normalizations

        def emit_pv(pv):
            p_t, pair, qq, j, t0, cw, oTA, oTB = pv
            off = (t0 - 4 * qq) * 128
            jlast = 4 * qq + 3
            nc.tensor.matmul(
                oTA[:, ds(off, cw)],
                vx_sb[j][:, ds(65 * (2 * pair), 65)],
                p_t[:, 0:cw],
                start=(j == 0), stop=(j == jlast), skip_group_check=True)
            nc.tensor.matmul(
                oTB[:, ds(off, cw)],
                vx_sb[j][:, ds(65 * (2 * pair + 1), 65)],
                p_t[:, ds(512, cw)],
                start=(j == 0), stop=(j == jlast), skip_group_check=True)

        def finish_quarter(pair, hr, qq, oT, csz):
            # Evacuate oT to SBUF right away so the PSUM banks free quickly;
            # then normalize from the copy, fully off the PE's path.
            nm = f"{pair}_{hr}_{qq}"
            num = p_r.tile([64, 512], BF16, tag="num", name=f"num{nm}")
            nc.vector.tensor_copy(num[:, :], oT[0:64, :])
            den = p_r.tile([1, 512], F32, tag="den", name=f"den{nm}")
            nc.vector.tensor_copy(den[:, :], oT[64:65, :])
            for c in range(512 // csz):
                r_t = p_r.tile([1, 512], F32, tag="r", name=f"r{nm}_{c}")
                nc.vector.reciprocal_approx_fast(r_t[:, 0:csz],
                                                 den[:, ds(csz * c, csz)])
                rb_t = p_r.tile([64, 512], F32, tag="rb", name=f"rb{nm}_{c}")
                nc.gpsimd.partition_broadcast(rb_t[:, 0:csz], r_t[:, 0:csz])
                nc.vector.tensor_mul(
                    aT_sb[pair][64 * hr:64 * hr + 64,
                                ds(512 * qq + csz * c, csz)],
                    num[:, ds(csz * c, csz)], rb_t[:, 0:csz])

        def flush_pend(inserts):
            if pend[0] is not None:
                emit_pv(pend[0])
                pend[0] = None
                for fin in finishes:
                    fin()
                finishes.clear()
            if inserts:
                ins_fn = inserts.pop(0)
                if ins_fn is not None:
                    ins_fn()

        def attn_quarter(pair, qq, inserts, last_csz=512):
            # q-tiles [4qq, 4qq+4) for heads (2*pair, 2*pair+1)
            qT, kT = qk_sb[pair], qk_sb[2 + pair]
            with nc.named_scope(f"attn_p{pair}q{qq}"):
                oTA = ps_o.tile([65, 512], F32, tag="oT",
                                name=f"oT{pair}_{qq}A")
                oTB = ps_o.tile([65, 512], F32, tag="oT",
                                name=f"oT{pair}_{qq}B")
                for j in range(4 * qq + 4):
                    t0 = max(j, 4 * qq)
                    cw = (4 * qq + 4 - t0) * 128
                    diag = (j >= 4 * qq)
                    s_t = ps_s.tile([128, 1024], F32, tag="s",
                                    name=f"s{pair}_{qq}_{j}")
                    # row-tiled K=64 scores: both heads concurrently
                    nc.tensor.matmul(
                        s_t[:, 0:cw], kT[0:64, ts(j, 128)],
                        qT[0:64, ds(t0 * 128, cw)],
                        start=True, stop=not diag, skip_group_check=True)
                    nc.tensor.matmul(
                        s_t[:, ds(512, cw)], kT[64:128, ts(j, 128)],
                        qT[64:128, ds(t0 * 128, cw)],
                        start=True, stop=not diag, skip_group_check=True)
                    if diag:
                        nc.tensor.matmul(
                            s_t[:, 0:128], neg_ident[:, :], upper01[:, :],
                            start=False, stop=True, skip_group_check=True)
                        nc.tensor.matmul(
                            s_t[:, 512:640], neg_ident[:, :], upper01[:, :],
                            start=False, stop=True, skip_group_check=True)
                    p_t = p_p.tile([128, 1024], BF16, tag="p")
                    if cw == 512:
                        nc.scalar.activation(p_t[:, :], s_t[:, :], AF.Exp,
                                             scale=SCALE)
                    else:
                        sv = s_t.rearrange("p (b n) -> p b n", n=512)
                        pv_ = p_t.rearrange("p (b n) -> p b n", n=512)
                        nc.scalar.activation(pv_[:, :, 0:cw], sv[:, :, 0:cw],
                                             AF.Exp, scale=SCALE)
                    flush_pend(inserts)
                    pend[0] = (p_t, pair, qq, j, t0, cw, oTA, oTB)
                for hr in range(2):
                    oT = oTA if hr == 0 else oTB
                    finishes.append(
                        lambda pair=pair, hr=hr, qq=qq, oT=oT, csz=last_csz:
                            finish_quarter(pair, hr, qq, oT, csz))

        # ---- emission: one interleaved stream ----
        with nc.named_scope("qk_proj_head"):
            for nh in range(2):
                proj_qk_half(0, nh)    # q01
            for nh in range(2):
                proj_qk_half(2, nh)    # k01
            proj_v4(0)                 # vx[0..3]
        # pair 0 quarters; meanwhile project q23/k23 + remaining v
        attn_quarter(0, 0, [lambda: proj_v4(1)])
        attn_quarter(0, 1, [lambda: proj_qk_half(1, 0),
                            lambda: proj_qk_half(1, 1),
                            lambda: proj_qk_half(3, 0),
                            lambda: proj_qk_half(3, 1)])
        attn_quarter(0, 2, [lambda: proj_v4(2), lambda: proj_v4(3)])
        attn_quarter(0, 3, [])
        # pair 1 quarters; meanwhile output projection of finished q ranges
        attn_quarter(1, 0, [])
        attn_quarter(1, 1, [None, lambda: proj_y(0), lambda: proj_y(1)])
        attn_quarter(1, 2, [None] + [lambda qt=qt: proj_y(qt)
                                     for qt in range(2, 8)])
        attn_quarter(1, 3, [None] + [lambda qt=qt: proj_y(qt)
                                     for qt in range(8, 12)],
                     last_csz=256)
        # tail: flush the last PV, then interleave the last normalizations
        # with the remaining y tiles
        emit_pv(pend[0])
        pend[0] = None
        with nc.named_scope("y_proj_tail"):
            for fin in finishes:
                fin()
            finishes.clear()
            for qt in range(12, NT):
                proj_y(qt)


def build():
    nc = bacc.Bacc("TRN2", target_bir_lowering=False, debug=False,
                   num_devices=NCORES)
    xT = nc.dram_tensor("xT", [D, S], BF16, kind="ExternalInput").ap()
    wqk = nc.dram_tensor("wqk", [D, 512], BF16, kind="ExternalInput").ap()
    wv = nc.dram_tensor("wv", [D, 256], BF16, kind="ExternalInput").ap()
    wout = nc.dram_tensor("wout", [256, D], BF16, kind="ExternalInput").ap()
    bqk = nc.dram_tensor("bqk", [512], F32, kind="ExternalInput").ap()
    bvb = nc.dram_tensor("bvb", [128, 256], F32, kind="ExternalInput").ap()
    ropeP = nc.dram_tensor("ropeP", [128, S], BF16, kind="ExternalInput").ap()
    ropeQ = nc.dram_tensor("ropeQ", [128, S], BF16, kind="ExternalInput").ap()
    y = nc.dram_tensor("y", [S, D], BF16, kind="ExternalOutput").ap()

    from contextlib import ExitStack
    with tile.TileContext(nc) as tc:
        with ExitStack() as ctx:
            _body(ctx, tc, (xT, wqk, wv, wout, bqk, bvb, ropeP, ropeQ), (y,))
    nc.compile()
    return nc


_EVEN_ODD = np.concatenate([np.arange(0, HD, 2), np.arange(1, HD, 2)])


def make_core_inputs(x, rope_cos, rope_sin, Wqkv, bqkv, Wout, bout, core):
    """Build the per-core device input map (numpy, host-side sharding)."""
    b, g = core // HPC, core % HPC
    heads = [HPC * g + i for i in range(HPC)]
    bf = ml_dtypes.bfloat16

    xT = np.ascontiguousarray(x[b].T).astype(bf)

    # wqk columns: [q01, q23, k01, k23]; within each head [evens, odds]
    qcols, kcols = [], []
    for h in heads:
        qcols.append(Wqkv[:, 0 * D + 64 * h + _EVEN_ODD])
        kcols.append(Wqkv[:, 1 * D + 64 * h + _EVEN_ODD])
    wqk_np = np.concatenate(
        [qcols[0], qcols[1], qcols[2], qcols[3],
         kcols[0], kcols[1], kcols[2], kcols[3]], axis=1)
    bq = [bqkv[0 * D + 64 * h + _EVEN_ODD] for h in heads]
    bk = [bqkv[1 * D + 64 * h + _EVEN_ODD] for h in heads]
    bqk_np = np.concatenate([bq[0], bq[1], bq[2], bq[3],
                             bk[0], bk[1], bk[2], bk[3]])

    wv_np = np.concatenate(
        [Wqkv[:, 2 * D + 64 * h:2 * D + 64 * h + 64] for h in heads], axis=1)
    bv = np.concatenate(
        [bqkv[2 * D + 64 * h:2 * D + 64 * h + 64] for h in heads])
    bvb_np = np.tile(bv[None, :], (128, 1)).astype(np.float32)

    wout_np = np.concatenate(
        [Wout[64 * h:64 * h + 64, :] for h in heads], axis=0)

    cosT = np.ascontiguousarray(rope_cos.T).astype(np.float32)  # [32, S]
    sinT = np.ascontiguousarray(rope_sin.T).astype(np.float32)
    ropeP_np = np.tile(np.concatenate([cosT, cosT], axis=0), (2, 1))
    ropeQ_np = np.tile(np.concatenate([-sinT, sinT], axis=0), (2, 1))

    return {
        "xT": xT,
        "wqk": np.ascontiguousarray(wqk_np).astype(bf),
        "wv": np.ascontiguousarray(wv_np).astype(bf),
        "wout": np.ascontiguousarray(wout_np).astype(bf),
        "bqk": bqk_np.astype(np.float32),
        "bvb": bvb_np,
        "ropeP": np.ascontiguousarray(ropeP_np).astype(bf),
        "ropeQ": np.ascontiguousarray(ropeQ_np).astype(bf),
    }


_NC_CACHE = None


def kernel(x, rope_cos, rope_sin, Wqkv, bqkv, Wout, bout):
    global _NC_CACHE, LAST_RESULTS
    x = np.asarray(x, dtype=np.float32)
    rope_cos = np.asarray(rope_cos, dtype=np.float32)
    rope_sin = np.asarray(rope_sin, dtype=np.float32)
    Wqkv = np.asarray(Wqkv, dtype=np.float32)
    bqkv = np.asarray(bqkv, dtype=np.float32)
    Wout = np.asarray(Wout, dtype=np.float32)
    bout = np.asarray(bout, dtype=np.float32)

    if _NC_CACHE is None:
        _NC_CACHE = build()
    nc = _NC_CACHE

    in_maps = [
        make_core_inputs(x, rope_cos, rope_sin, Wqkv, bqkv, Wout, bout, c)
        for c in range(NCORES)
    ]
    res = run_bass_kernel_spmd(nc, in_maps, core_ids=list(range(NCORES)),
                               trace=TRACE)
    LAST_RESULTS = res

    out = np.zeros((B, S, D), dtype=np.float32)
    for c in range(NCORES):
        out[c // HPC] += np.asarray(res.results[c]["y"]).astype(np.float32)
    out += bout[None, None, :]
    return out
